# revision 16
# baseline (speedup 1.0000x reference)
"""Trainium2 Bass kernel for nn_EmbeddingLayer (embedding lookup + dense + positional encoding).

Computation (per reference):
    x = emb_table[tokens] * sqrt(512)          [B, F, E]
    x = x.reshape(B, F*E) @ W + b              [B, M]
    out = x[:, None, :] + pe[:128, :]          [B, S, M]   (1 GiB f32 output)

Strategy: data-parallel over batch across 8 cores (512 rows each); the
74 MB table and the dense weight are replicated.  Per core:
  - indirect-DMA gather, one token column per instruction (HW honors one
    index per partition), 256 gathers of 128 rows x 128 B
  - PE transpose (f32 fast transpose mode) -> xT cast to bf16, bf16
    matmul against pre-scaled bf16 W (scale/bias folded on host)
  - pe rows broadcast across partitions with a ones(1x128) bf16 matmul
    on PE; the f32 psum result is consumed directly by the DVE add
    (no psum->SBUF copy)
  - DVE broadcast-add y[b,m] + pe[s,m] -> f32 out tiles
  - 1 MiB HWDGE stores of the 128 MiB per-core output (the roofline)
Loop order streams per batch-chunk so adds/stores begin as soon as the
first chunk's gathers land.
"""

import sys

import numpy as np

if "/opt/trn_rl_repo" not in sys.path:
    sys.path.insert(0, "/opt/trn_rl_repo")

BATCH = 4096
FEATURES = 64
VOCAB = 580000
EMB = 32
MODELS = 512
SEQ = 128
N_CORES = 8
BS = BATCH // N_CORES  # 512 rows per core

P = 128                 # partitions
NB = BS // P            # 4 batch chunks per core
KC = (FEATURES * EMB) // P  # 16 k-chunks of 128

_MODULE_CACHE = {}


def _positional_encoding(position, d_model):
    # mirror of reference._positional_encoding, in numpy f32
    pos = np.arange(position, dtype=np.float32)[:, None]
    i = np.arange(d_model, dtype=np.float32)[None, :]
    angle_rates = 1.0 / np.power(
        10000.0, (2.0 * np.floor(i / 2.0)) / np.float32(d_model)
    )
    angles = (pos * angle_rates).astype(np.float32)
    even = (np.arange(d_model) % 2 == 0)[None, :]
    pe = np.where(even, np.sin(angles), np.cos(angles)).astype(np.float32)
    return pe  # [S, M]


def build_module():
    """Build + compile the per-core Bass module (identical program on all cores)."""
    if "nc" in _MODULE_CACHE:
        return _MODULE_CACHE["nc"]

    from contextlib import ExitStack

    import concourse.bass as bass
    import concourse.tile as tile
    from concourse import bacc, mybir

    f32 = mybir.dt.float32
    bf16 = mybir.dt.bfloat16
    i32 = mybir.dt.int32

    nc = bacc.Bacc("TRN2", target_bir_lowering=False, debug=False,
                   num_devices=N_CORES)

    tok = nc.dram_tensor("tok", [BS, FEATURES], i32, kind="ExternalInput").ap()
    emb = nc.dram_tensor("emb", [VOCAB, EMB], f32, kind="ExternalInput").ap()
    w = nc.dram_tensor("w", [FEATURES * EMB, MODELS], bf16, kind="ExternalInput").ap()
    pe = nc.dram_tensor("pe", [SEQ, MODELS], bf16, kind="ExternalInput").ap()
    ident = nc.dram_tensor("ident", [P, P], f32, kind="ExternalInput").ap()
    ones = nc.dram_tensor("ones", [1, P], bf16, kind="ExternalInput").ap()
    out = nc.dram_tensor("out", [BS, SEQ, MODELS], f32, kind="ExternalOutput").ap()

    with tile.TileContext(nc) as tc, ExitStack() as ctx:
        const = ctx.enter_context(tc.tile_pool(name="const", bufs=1))
        tok_pool = ctx.enter_context(tc.tile_pool(name="tok", bufs=NB))
        x_pool = ctx.enter_context(tc.tile_pool(name="x", bufs=3))
        xT_pool = ctx.enter_context(tc.tile_pool(name="xT", bufs=2))
        y_pool = ctx.enter_context(tc.tile_pool(name="y", bufs=2))
        perow_pool = ctx.enter_context(tc.tile_pool(name="perow", bufs=8))
        out_pool = ctx.enter_context(tc.tile_pool(name="outp", bufs=3))
        psum_t = ctx.enter_context(tc.tile_pool(name="pst", bufs=1, space="PSUM"))
        psum_y = ctx.enter_context(tc.tile_pool(name="psy", bufs=1, space="PSUM"))
        psum_p = ctx.enter_context(tc.tile_pool(name="psp", bufs=3, space="PSUM"))

        # token loads first (gathers need them immediately); the rest of the
        # inputs ride the ScalarE HWDGE ring behind them
        tok_sbs = []
        for c in range(NB):
            tok_sb = tok_pool.tile([P, FEATURES], i32)
            nc.scalar.dma_start(tok_sb[:], tok[c * P:(c + 1) * P, :])
            tok_sbs.append(tok_sb)

        id_sb = const.tile([P, P], f32)
        nc.scalar.dma_start(id_sb[:], ident[:])
        ones_sb = const.tile([1, P], bf16)
        nc.scalar.dma_start(ones_sb[:], ones[:])
        w_sb = const.tile([P, KC * MODELS], bf16)
        nc.scalar.dma_start(
            w_sb[:].rearrange("p (kc m) -> p kc m", kc=KC),
            w.rearrange("(kc p) m -> p kc m", p=P),
        )

        # Software-pipelined windows: window w gathers chunk w (if any) while
        # adding/storing chunk w-1.  Emission is interleaved at ~10 us
        # granularity so every engine's in-order stream matches real-time
        # data availability.  Stores ride the same SWDGE queue as the
        # gathers (HWDGE queues starve while SWDGE traffic flows).
        y_sbs = {}

        def emit_gather_half(c, step, h, x_sb, tok_sb):
            # 8 gathers = 2 k-chunks
            for kc in (4 * step + 2 * h, 4 * step + 2 * h + 1):
                for jf in range(4):
                    f = 4 * kc + jf
                    nc.gpsimd.indirect_dma_start(
                        out=x_sb[:, f * EMB:(f + 1) * EMB],
                        out_offset=None,
                        in_=emb[:],
                        in_offset=bass.IndirectOffsetOnAxis(
                            ap=tok_sb[:, f:f + 1], axis=0
                        ),
                    )

        def emit_transpose_half(step, h, x_sb, pt):
            for jj in range(2):
                kc = 4 * step + 2 * h + jj
                nc.tensor.transpose(
                    out=pt[:, (2 * h + jj) * P:(2 * h + jj + 1) * P],
                    in_=x_sb[:, kc * P:(kc + 1) * P],
                    identity=id_sb[:],
                )

        for w in range(NB + 1):
            gc = w if w < NB else None      # chunk being gathered
            ac = w - 1 if w >= 1 else None  # chunk being added/stored
            if gc is not None:
                x_sb = x_pool.tile([P, FEATURES * EMB], f32)
                xT_sb = xT_pool.tile([P, KC * P], bf16)
                tok_sb = tok_sbs[gc]
            y_sb = y_sbs.get(ac)

            perow_fifo = []

            def load_perows(step):
                for q in range(8):
                    s0 = step * 32 + q * 4
                    perow = perow_pool.tile([1, 4 * MODELS], bf16)
                    nc.scalar.dma_start(
                        perow[:].rearrange("p (g m) -> p g m", g=4),
                        pe[s0:s0 + 4, :].unsqueeze(0),
                    )
                    perow_fifo.append(perow)

            if ac is not None:
                load_perows(0)  # prefetch a full step ahead

            pending_store = []
            for step in range(4):
                if ac is not None and step < 3:
                    load_perows(step + 1)
                perows = perow_fifo[step * 8:(step + 1) * 8]

                if gc is not None:
                    pt = psum_t.tile([P, 4 * P], f32)
                if ac is not None:
                    ot = out_pool.tile([P, 32 * MODELS], bf16)
                for h in range(2):
                    if gc is not None:
                        emit_gather_half(gc, step, h, x_sb, tok_sb)
                    if ac is not None:
                        for pq in range(8):  # 8 pp tiles x 2 seq rows
                            perow = perows[h * 4 + pq // 2]
                            pp = psum_p.tile([P, 2 * MODELS], f32)
                            for u in range(2):
                                r0 = ((pq % 2) * 2 + u) * MODELS
                                nc.tensor.matmul(
                                    pp[:, u * MODELS:(u + 1) * MODELS],
                                    lhsT=ones_sb[:],
                                    rhs=perow[:, r0:r0 + MODELS],
                                    start=True,
                                    stop=True,
                                )
                            o0 = (h * 8 + pq) * 2 * MODELS
                            nc.vector.tensor_tensor(
                                out=ot[:, o0:o0 + 2 * MODELS]
                                    .rearrange("p (g m) -> p g m", g=2),
                                in0=y_sb[:].unsqueeze(1)
                                    .to_broadcast([P, 2, MODELS]),
                                in1=pp[:].rearrange("p (g m) -> p g m", g=2),
                                op=mybir.AluOpType.add,
                            )
                    if gc is not None:
                        emit_transpose_half(step, h, x_sb, pt)
                if ac is not None:
                    s0 = step * 32
                    nc.gpsimd.dma_start(
                        out[ac * P:(ac + 1) * P, s0:s0 + 32, :],
                        ot[:].rearrange("p (g m) -> p g m", g=32),
                    )
                if gc is not None:
                    nc.vector.tensor_copy(
                        xT_sb[:, step * 4 * P:(step + 1) * 4 * P], pt[:]
                    )

            if gc is not None:
                py = psum_y.tile([P, MODELS], f32)
                for kc in range(KC):
                    nc.tensor.matmul(
                        py[:],
                        lhsT=xT_sb[:, kc * P:(kc + 1) * P],
                        rhs=w_sb[:, kc * MODELS:(kc + 1) * MODELS],
                        start=(kc == 0),
                        stop=(kc == KC - 1),
                    )
                y_new = y_pool.tile([P, MODELS], f32)
                nc.scalar.copy(y_new[:], py[:])
                y_sbs[gc] = y_new

    nc.compile()
    _MODULE_CACHE["nc"] = nc
    return nc


def make_in_maps(tokens, emb_table, W, b):
    import ml_dtypes

    tokens = np.ascontiguousarray(np.asarray(tokens, dtype=np.int32))
    emb_table = np.ascontiguousarray(np.asarray(emb_table, dtype=np.float32))
    W = np.asarray(W, dtype=np.float32)
    b = np.asarray(b, dtype=np.float32)

    wp = np.ascontiguousarray(
        (W * np.float32(np.sqrt(np.float32(MODELS)))).astype(ml_dtypes.bfloat16)
    )
    peb = np.ascontiguousarray(
        (_positional_encoding(SEQ, MODELS) + b[None, :].astype(np.float32))
        .astype(ml_dtypes.bfloat16)
    )
    ident = np.eye(P, dtype=np.float32)
    ones = np.ones((1, P), dtype=ml_dtypes.bfloat16)

    in_maps = []
    for c in range(N_CORES):
        in_maps.append({
            "tok": tokens[c * BS:(c + 1) * BS],
            "emb": emb_table,
            "w": wp,
            "pe": peb,
            "ident": ident,
            "ones": ones,
        })
    return in_maps


def run(tokens, emb_table, W, b, trace=False):
    """Run on 8 NeuronCores; returns (full_output, BassKernelResults)."""
    from concourse import bass_utils

    nc = build_module()
    in_maps = make_in_maps(tokens, emb_table, W, b)
    res = bass_utils.run_bass_kernel_spmd(
        nc, in_maps, core_ids=list(range(N_CORES)), trace=trace
    )
    outs = [r["out"] for r in res.results]
    full = np.concatenate(outs, axis=0)
    return full, res


def kernel(tokens, emb_table, W, b):
    full, _ = run(tokens, emb_table, W, b, trace=False)
    return full


# revision 17
# speedup vs baseline: 1.1486x; 1.1486x over previous
"""Trainium2 Bass kernel for nn_EmbeddingLayer (embedding lookup + dense + positional encoding).

Computation (per reference):
    x = emb_table[tokens] * sqrt(512)          [B, F, E]
    x = x.reshape(B, F*E) @ W + b              [B, M]
    out = x[:, None, :] + pe[:128, :]          [B, S, M]   (1 GiB f32 output)

Strategy: data-parallel over batch across 8 cores (512 rows each); the
74 MB table and the dense weight are replicated.  Per core:
  - indirect-DMA gather, one token column per instruction (HW honors one
    index per partition), 256 gathers of 128 rows x 128 B
  - PE transpose (f32 fast transpose mode) -> xT cast to bf16, bf16
    matmul against pre-scaled bf16 W (scale/bias folded on host)
  - pe rows broadcast across partitions with a ones(1x128) bf16 matmul
    on PE; the f32 psum result is consumed directly by the DVE add
    (no psum->SBUF copy)
  - DVE broadcast-add y[b,m] + pe[s,m] -> f32 out tiles
  - 1 MiB HWDGE stores of the 128 MiB per-core output (the roofline)
Loop order streams per batch-chunk so adds/stores begin as soon as the
first chunk's gathers land.
"""

import sys

import numpy as np

if "/opt/trn_rl_repo" not in sys.path:
    sys.path.insert(0, "/opt/trn_rl_repo")

BATCH = 4096
FEATURES = 64
VOCAB = 580000
EMB = 32
MODELS = 512
SEQ = 128
N_CORES = 8
BS = BATCH // N_CORES  # 512 rows per core

P = 128                 # partitions
NB = BS // P            # 4 batch chunks per core
KC = (FEATURES * EMB) // P  # 16 k-chunks of 128

_MODULE_CACHE = {}


def _positional_encoding(position, d_model):
    # mirror of reference._positional_encoding, in numpy f32
    pos = np.arange(position, dtype=np.float32)[:, None]
    i = np.arange(d_model, dtype=np.float32)[None, :]
    angle_rates = 1.0 / np.power(
        10000.0, (2.0 * np.floor(i / 2.0)) / np.float32(d_model)
    )
    angles = (pos * angle_rates).astype(np.float32)
    even = (np.arange(d_model) % 2 == 0)[None, :]
    pe = np.where(even, np.sin(angles), np.cos(angles)).astype(np.float32)
    return pe  # [S, M]


def build_module():
    """Build + compile the per-core Bass module (identical program on all cores)."""
    if "nc" in _MODULE_CACHE:
        return _MODULE_CACHE["nc"]

    from contextlib import ExitStack

    import concourse.bass as bass
    import concourse.tile as tile
    from concourse import bacc, mybir

    f32 = mybir.dt.float32
    bf16 = mybir.dt.bfloat16
    i32 = mybir.dt.int32

    nc = bacc.Bacc("TRN2", target_bir_lowering=False, debug=False,
                   num_devices=N_CORES)

    tok = nc.dram_tensor("tok", [BS, FEATURES], i32, kind="ExternalInput").ap()
    emb = nc.dram_tensor("emb", [VOCAB, EMB], f32, kind="ExternalInput").ap()
    w = nc.dram_tensor("w", [FEATURES * EMB, MODELS], bf16, kind="ExternalInput").ap()
    pe = nc.dram_tensor("pe", [SEQ, MODELS], bf16, kind="ExternalInput").ap()
    ident = nc.dram_tensor("ident", [P, P], f32, kind="ExternalInput").ap()
    ones = nc.dram_tensor("ones", [1, P], bf16, kind="ExternalInput").ap()
    out = nc.dram_tensor("out", [BS, SEQ, MODELS], f32, kind="ExternalOutput").ap()

    with tile.TileContext(nc) as tc, ExitStack() as ctx:
        const = ctx.enter_context(tc.tile_pool(name="const", bufs=1))
        tok_pool = ctx.enter_context(tc.tile_pool(name="tok", bufs=NB))
        x_pool = ctx.enter_context(tc.tile_pool(name="x", bufs=3))
        xT_pool = ctx.enter_context(tc.tile_pool(name="xT", bufs=2))
        y_pool = ctx.enter_context(tc.tile_pool(name="y", bufs=2))
        perow_pool = ctx.enter_context(tc.tile_pool(name="perow", bufs=8))
        out_pool = ctx.enter_context(tc.tile_pool(name="outp", bufs=6))
        psum_t = ctx.enter_context(tc.tile_pool(name="pst", bufs=1, space="PSUM"))
        psum_y = ctx.enter_context(tc.tile_pool(name="psy", bufs=1, space="PSUM"))
        psum_p = ctx.enter_context(tc.tile_pool(name="psp", bufs=3, space="PSUM"))

        # token loads first (gathers need them immediately); the rest of the
        # inputs ride the ScalarE HWDGE ring behind them
        tok_sbs = []
        for c in range(NB):
            tok_sb = tok_pool.tile([P, FEATURES], i32)
            nc.scalar.dma_start(tok_sb[:], tok[c * P:(c + 1) * P, :])
            tok_sbs.append(tok_sb)

        id_sb = const.tile([P, P], f32)
        nc.scalar.dma_start(id_sb[:], ident[:])
        ones_sb = const.tile([1, P], bf16)
        nc.scalar.dma_start(ones_sb[:], ones[:])
        w_sb = const.tile([P, KC * MODELS], bf16)
        nc.scalar.dma_start(
            w_sb[:].rearrange("p (kc m) -> p kc m", kc=KC),
            w.rearrange("(kc p) m -> p kc m", p=P),
        )

        # Software-pipelined windows: window w gathers chunk w (if any) while
        # adding/storing chunk w-1.  Emission is interleaved at ~10 us
        # granularity so every engine's in-order stream matches real-time
        # data availability.  Stores ride the same SWDGE queue as the
        # gathers (HWDGE queues starve while SWDGE traffic flows).
        y_sbs = {}

        def emit_gather_half(c, step, h, x_sb, tok_sb):
            # 8 gathers = 2 k-chunks
            for kc in (4 * step + 2 * h, 4 * step + 2 * h + 1):
                for jf in range(4):
                    f = 4 * kc + jf
                    nc.gpsimd.indirect_dma_start(
                        out=x_sb[:, f * EMB:(f + 1) * EMB],
                        out_offset=None,
                        in_=emb[:],
                        in_offset=bass.IndirectOffsetOnAxis(
                            ap=tok_sb[:, f:f + 1], axis=0
                        ),
                    )

        def emit_transpose_half(step, h, x_sb, pt):
            for jj in range(2):
                kc = 4 * step + 2 * h + jj
                nc.tensor.transpose(
                    out=pt[:, (2 * h + jj) * P:(2 * h + jj + 1) * P],
                    in_=x_sb[:, kc * P:(kc + 1) * P],
                    identity=id_sb[:],
                )

        for w in range(NB + 1):
            gc = w if w < NB else None      # chunk being gathered
            ac = w - 1 if w >= 1 else None  # chunk being added/stored
            if gc is not None:
                x_sb = x_pool.tile([P, FEATURES * EMB], f32)
                xT_sb = xT_pool.tile([P, KC * P], bf16)
                tok_sb = tok_sbs[gc]
            y_sb = y_sbs.get(ac)

            perow_fifo = []

            def load_perows(step):
                for q in range(8):
                    s0 = step * 32 + q * 4
                    perow = perow_pool.tile([1, 4 * MODELS], bf16)
                    nc.scalar.dma_start(
                        perow[:].rearrange("p (g m) -> p g m", g=4),
                        pe[s0:s0 + 4, :].unsqueeze(0),
                    )
                    perow_fifo.append(perow)

            if ac is not None:
                load_perows(0)  # prefetch a full step ahead

            pending_store = []
            for step in range(4):
                if ac is not None and step < 3:
                    load_perows(step + 1)
                perows = perow_fifo[step * 8:(step + 1) * 8]

                if gc is not None:
                    pt = psum_t.tile([P, 4 * P], f32)
                for h in range(2):          # one 'go' output tile per half-step
                    if gc is not None:
                        emit_gather_half(gc, step, h, x_sb, tok_sb)
                    if ac is not None:
                        go = step * 2 + h
                        ot = out_pool.tile([P, 16 * MODELS], bf16)
                        for pq in range(8):  # 8 pp tiles x 2 seq rows
                            perow = perows[h * 4 + pq // 2]
                            pp = psum_p.tile([P, 2 * MODELS], f32)
                            for u in range(2):
                                r0 = ((pq % 2) * 2 + u) * MODELS
                                nc.tensor.matmul(
                                    pp[:, u * MODELS:(u + 1) * MODELS],
                                    lhsT=ones_sb[:],
                                    rhs=perow[:, r0:r0 + MODELS],
                                    start=True,
                                    stop=True,
                                )
                            o0 = pq * 2 * MODELS
                            nc.vector.tensor_tensor(
                                out=ot[:, o0:o0 + 2 * MODELS]
                                    .rearrange("p (g m) -> p g m", g=2),
                                in0=y_sb[:].unsqueeze(1)
                                    .to_broadcast([P, 2, MODELS]),
                                in1=pp[:].rearrange("p (g m) -> p g m", g=2),
                                op=mybir.AluOpType.add,
                            )
                        s0 = go * 16
                        nc.gpsimd.dma_start(
                            out[ac * P:(ac + 1) * P, s0:s0 + 16, :],
                            ot[:].rearrange("p (g m) -> p g m", g=16),
                        )
                    if gc is not None:
                        emit_transpose_half(step, h, x_sb, pt)
                if gc is not None:
                    nc.vector.tensor_copy(
                        xT_sb[:, step * 4 * P:(step + 1) * 4 * P], pt[:]
                    )

            if gc is not None:
                py = psum_y.tile([P, MODELS], f32)
                for kc in range(KC):
                    nc.tensor.matmul(
                        py[:],
                        lhsT=xT_sb[:, kc * P:(kc + 1) * P],
                        rhs=w_sb[:, kc * MODELS:(kc + 1) * MODELS],
                        start=(kc == 0),
                        stop=(kc == KC - 1),
                    )
                y_new = y_pool.tile([P, MODELS], f32)
                nc.scalar.copy(y_new[:], py[:])
                y_sbs[gc] = y_new

    nc.compile()
    _MODULE_CACHE["nc"] = nc
    return nc


def make_in_maps(tokens, emb_table, W, b):
    import ml_dtypes

    tokens = np.ascontiguousarray(np.asarray(tokens, dtype=np.int32))
    emb_table = np.ascontiguousarray(np.asarray(emb_table, dtype=np.float32))
    W = np.asarray(W, dtype=np.float32)
    b = np.asarray(b, dtype=np.float32)

    wp = np.ascontiguousarray(
        (W * np.float32(np.sqrt(np.float32(MODELS)))).astype(ml_dtypes.bfloat16)
    )
    peb = np.ascontiguousarray(
        (_positional_encoding(SEQ, MODELS) + b[None, :].astype(np.float32))
        .astype(ml_dtypes.bfloat16)
    )
    ident = np.eye(P, dtype=np.float32)
    ones = np.ones((1, P), dtype=ml_dtypes.bfloat16)

    in_maps = []
    for c in range(N_CORES):
        in_maps.append({
            "tok": tokens[c * BS:(c + 1) * BS],
            "emb": emb_table,
            "w": wp,
            "pe": peb,
            "ident": ident,
            "ones": ones,
        })
    return in_maps


def run(tokens, emb_table, W, b, trace=False):
    """Run on 8 NeuronCores; returns (full_output, BassKernelResults)."""
    from concourse import bass_utils

    nc = build_module()
    in_maps = make_in_maps(tokens, emb_table, W, b)
    res = bass_utils.run_bass_kernel_spmd(
        nc, in_maps, core_ids=list(range(N_CORES)), trace=trace
    )
    outs = [r["out"] for r in res.results]
    full = np.concatenate(outs, axis=0)
    return full, res


def kernel(tokens, emb_table, W, b):
    full, _ = run(tokens, emb_table, W, b, trace=False)
    return full
